# revision 76
# baseline (speedup 1.0000x reference)
"""Trainium2 Bass kernel: MultiHeadAttention over [2, 512, 64, 64] images.

Sharding: 8 cores = (2 batches) x (4 head-pairs). Each core computes 2 of the
8 attention heads for one batch plus a partial output projection over its 128
input channels; the host sums the 4 partial projections per batch and adds
the output bias (the unshard step for a contraction-dim tensor-parallel
split).

Per-core pipeline (all L=4096 positions, everything fp16 into the PE —
fp8/DoubleRow fails the 2e-2 gate: the output is an attenuated residual of
near-uniform attention averaging, so per-weight noise e contributes ~2.3e
of relative error; the budget only allows e <~ 0.5%):
  QKV:  Q/K in [c=128, l] layout (2 heads x 64 dk-channels on partitions),
        V transposed on the PE into VT [s, c] with an interleaved ones
        column. Shares the attention phase's PSUM pools (a separate pool
        would insert a teardown barrier); per-ktile x DMAs on two queues.
  Attn: S^T tiles [s=128, t=512] via K=64 matmuls that the hardware pairs
        into concurrent h0/h64 row groups (full PE array on QK). The
        exp(score) softmax weights are produced by TWO engines in parallel
        (scores are O(+-11) raw = O(+-1.4) scaled, so softmax needs no max
        subtraction):
          - ACT: hardware Exp spline straight out of PSUM,
          - DVE: EXP_PSQ4_ANT, a custom 8-stage microprogram computing
            (1 + k*s*(s^2+a*s+b))^4 ~ exp(s/8) to ~1.5e-3 in ONE pass.
        ACT_UNITS tunes the per-engine unit split. AV matmuls run 2 s-tiles
        behind QK so the in-order PE queue never waits on an exp; the last
        two AVs of each t-tile cross into the next tile's s-loop.
  Norm: ones-row in the AV lhsT accumulates the softmax denominator in PSUM
        row 64; RECIPROCAL_APPROX_FAST (custom DVE, ~51 ULP) on an SBUF
        staging copy, fp16 partition-broadcast via PE outer products, one
        fused multiply. The whole chain + the output projection defer into
        the next t-tile's s-loop as (due_s, closure) pops so single PE ops
        with fresh deps never stall the in-order PE queue.
  Proj: partial Wp projection, fp16 results DMA'd straight out; the host
        adds the output bias while summing partials.
"""

import math
import numpy as np

B, C, HH, WW = 2, 512, 64, 64
L = HH * WW          # 4096
NH, DK = 8, 64
SCALE = 1.0 / math.sqrt(DK)
NCORES = 8

TT = 512             # t-tile width (columns per attention tile)
NT = L // TT         # 8 t-tiles
NS = L // 128        # 32 s-tiles
KT = C // 128        # 4 contraction tiles for projections

# exp(s/8) ~ (1 + EK*s*(s^2 + EA*s + EB))^4 on s in [-12.5, 12.5]
# (max rel err 1.5e-3 incl. fp16 store; fitted in /tmp/fit_exp.py)
EA = 101.39437425803705
EB = 6422.57504081101
EK = 4.8710393819014345e-06

# of every 32 s-tiles, this many exp units go to ACT; rest to the DVE op
ACT_UNITS = 23

_BUILT = {}
_EXP_OP = None


def _get_exp_op():
    """Register the custom DVE op (documented extension point: a DveOp in
    dve_ops.OPS with a pinned uops_sha; the per-NEFF table is generated by
    bass_utils.dve_table_for_ops from these entries)."""
    global _EXP_OP
    if _EXP_OP is not None:
        return _EXP_OP
    import concourse.dve_ops as dve_ops
    from concourse.dve_spec import Spec, Src0, C0, C1, C2, One, sq

    body = sq(sq(((Src0 + C0) * Src0 + C1) * Src0 * C2 + One))

    def ref(in0, in1, s0, s1, imm2):
        x = in0.astype(np.float32)
        p = (1.0 + imm2 * x * ((x + s0) * x + s1)).astype(np.float32)
        return (p * p) * (p * p)

    op = dve_ops.DveOp("EXP_PSQ4_ANT", Spec(body=body, reference=ref),
                       subdim=False,
                       uops_sha={"v3": "3c513f5b3b2b5d19"})
    if op.name not in dve_ops._SUB_OPCODE_FOR_NAME:
        dve_ops._SUB_OPCODE_FOR_NAME[op.name] = (
            max(dve_ops._SUB_OPCODE_FOR_NAME.values()) + 1)
        dve_ops.OPS.append(op)
        dve_ops.CUSTOM_DVE_SPECS[op.name] = op.spec
    _EXP_OP = op
    return op


def _build(l=L):
    import concourse.bacc as bacc
    import concourse.tile as tile
    import concourse.mybir as mybir
    from concourse.masks import make_identity
    from contextlib import ExitStack

    exp_op = _get_exp_op()

    nt = l // TT
    ns = l // 128
    f32 = mybir.dt.float32
    f16 = mybir.dt.float16
    f32r = mybir.dt.float32r
    Exp = mybir.ActivationFunctionType.Exp
    add = mybir.AluOpType.add

    # s-tiles handled by ACT (evenly interleaved with the DVE ones)
    act_pat = [(s * ACT_UNITS) % ns < ACT_UNITS for s in range(ns)]

    nc = bacc.Bacc("TRN2", target_bir_lowering=False, debug=False,
                   num_devices=NCORES)

    # w_all is host-packed in the SBUF tile layout [p, i, kt, o] so ONE
    # contiguous-per-partition DMA loads all projection weights
    x = nc.dram_tensor("x", [KT, 128, l], f16, kind="ExternalInput").ap()
    w_all = nc.dram_tensor("w_all", [128, 3, KT, 128], f16,
                           kind="ExternalInput").ap()
    b_all = nc.dram_tensor("b_all", [128, 3], f32, kind="ExternalInput").ap()
    wp = nc.dram_tensor("wp", [128, C], f16, kind="ExternalInput").ap()
    o2 = nc.dram_tensor("ones2", [1, 2, 128], f32, kind="ExternalInput").ap()
    out = nc.dram_tensor("out", [C, l], f16, kind="ExternalOutput").ap()

    with tile.TileContext(nc) as tc, ExitStack() as ctx:
        persist = ctx.enter_context(tc.tile_pool(name="persist", bufs=1))
        e_pool = ctx.enter_context(tc.tile_pool(name="e", bufs=6))
        o_pool = ctx.enter_context(tc.tile_pool(name="o", bufs=2))
        z_pool = ctx.enter_context(tc.tile_pool(name="z", bufs=2))
        res_pool = ctx.enter_context(tc.tile_pool(name="res", bufs=3))

        # ---- weights: fp16 straight from DRAM; per-projection DMAs so the
        # first QKV matmul only waits for the Wq chunk ----
        w_r = persist.tile([128, 3, KT, 128], f16, tag="wr")
        for i in range(3):
            nc.sync.dma_start(out=w_r[:, i, :, :], in_=w_all[:, i, :, :])
        wp_r = persist.tile([128, C], f16, tag="wpr")
        nc.sync.dma_start(out=wp_r, in_=wp)

        bias_sb = persist.tile([128, 3], f32, tag="bias")
        nc.sync.dma_start(out=bias_sb, in_=b_all)

        ident = persist.tile([128, 128], f16, tag="ident")
        make_identity(nc, ident)

        # half-ones lhsT rows: two accumulating outer products broadcast
        # rz row 0 -> out partitions 0:64 and rz row 1 -> partitions 64:128
        ones2_f = persist.tile([1, 2, 128], f32, tag="ones2f")
        nc.sync.dma_start(out=ones2_f, in_=o2)
        # fp16 operands: f32r 1-partition matmuls lower to 4-pass fp32-HIGH
        ones_a = persist.tile([1, 128], f16, tag="onesa")
        ones_b = persist.tile([1, 128], f16, tag="onesb")
        nc.vector.tensor_copy(ones_a, ones2_f[0:1, 0, :])
        nc.vector.tensor_copy(ones_b, ones2_f[0:1, 1, :])

        # ---- persistent activations ----
        q_sb = persist.tile([128, l], f16, tag="q")
        k_sb = persist.tile([128, l], f16, tag="k")
        vt_sb = persist.tile([128, ns, 130], f16, tag="vt")
        # interleaved ones columns -> softmax denominator rows in AV psum
        ones_col = persist.tile([128, ns, 1], f32, tag="onescol")
        nc.vector.memset(ones_col, 1.0)
        nc.vector.tensor_copy(vt_sb[:, :, 64:65], ones_col)
        nc.vector.tensor_copy(vt_sb[:, :, 129:130], ones_col)

        # shared PSUM pools for both phases (a separate QKV psum pool would
        # insert a multi-microsecond teardown barrier before attention)
        x_pool = ctx.enter_context(tc.tile_pool(name="xs", bufs=3))
        v_pool = ctx.enter_context(tc.tile_pool(name="vsb", bufs=2))
        st_pool = ctx.enter_context(
            tc.tile_pool(name="stps", bufs=2, space="PSUM"))
        av_pool = ctx.enter_context(
            tc.tile_pool(name="avps", bufs=3, space="PSUM"))
        pr_pool = ctx.enter_context(
            tc.tile_pool(name="prps", bufs=1, space="PSUM"))

        # ================= QKV projections + V transpose =================
        if True:
            Copy = mybir.ActivationFunctionType.Copy
            for n in range(nt):
                nsl = slice(n * TT, (n + 1) * TT)
                # per-ktile x DMAs alternating between two queues: the kt=0
                # matmul starts as soon as its 256KB chunk lands
                x_r = x_pool.tile([128, KT, TT], f16, tag="x")
                x_v = x.rearrange("kt p l -> p kt l")
                for kt in range(KT):
                    eng = nc.scalar if kt % 2 == 0 else nc.gpsimd
                    eng.dma_start(out=x_r[:, kt:kt + 1, :],
                                  in_=x_v[:, kt:kt + 1, nsl])

                qk_ps = st_pool.tile([128, 2, TT], f32, tag="st", name="qkps")
                v_ps = av_pool.tile([128, TT], f32, tag="av", name="vps")
                for wi, ps in enumerate((qk_ps[:, 0, :], qk_ps[:, 1, :], v_ps)):
                    for kt in range(KT):
                        nc.tensor.matmul(ps, w_r[:, wi, kt, :], x_r[:, kt, :],
                                         start=(kt == 0), stop=(kt == KT - 1))

                nc.vector.tensor_scalar(q_sb[:, nsl], qk_ps[:, 0, :],
                                        bias_sb[:, 0:1], None, add)
                nc.vector.tensor_scalar(k_sb[:, nsl], qk_ps[:, 1, :],
                                        bias_sb[:, 1:2], None, add)
                v_sb = v_pool.tile([128, TT], f16, tag="v")
                nc.vector.tensor_scalar(v_sb, v_ps, bias_sb[:, 2:3], None, add)

                # transpose V tile: 4 PE transposes -> [s, c] in psum
                tp = av_pool.tile([128, TT], f16, tag="av", name="tp")
                for j in range(4):
                    nc.tensor.transpose(tp[:, j * 128:(j + 1) * 128],
                                        v_sb[:, j * 128:(j + 1) * 128], ident)
                tp_v = tp.rearrange("p (j c) -> p j c", j=4)
                ssl = slice(4 * n, 4 * n + 4)
                # ACT is idle during the QKV phase; DVE is its co-bottleneck
                nc.scalar.activation(vt_sb[:, ssl, 0:64], tp_v[:, :, 0:64],
                                     Copy)
                nc.scalar.activation(vt_sb[:, ssl, 65:129], tp_v[:, :, 64:128],
                                     Copy)

        # ========================= attention =========================
        if True:

            # deferred work from the previous t-tile: list of (due_s, fn);
            # popped inside the next tile's s-loop so the tail AV matmuls,
            # normalize chain and projection never stall the in-order PE
            # queue (its first QK ops fill the exp/DVE latency)
            pending = []
            from concourse.dve_ops import (RECIP_APPROX_FAST_CONSTS,
                                           RECIPROCAL_APPROX_FAST)
            rcc = RECIP_APPROX_FAST_CONSTS
            Copy = mybir.ActivationFunctionType.Copy

            for t in range(nt):
                tsl = slice(t * TT, (t + 1) * TT)
                av = [None, None]
                e_tiles = {}

                for s in range(ns):
                    while pending and pending[0][0] <= s:
                        pending.pop(0)[1]()
                    if s == 2:
                        # lazy alloc: the previous tile's AV psum reads must
                        # be emitted before these slots are reclaimed
                        av[0] = av_pool.tile([128, TT], f32, tag="av",
                                             name="av0")
                        av[1] = av_pool.tile([128, TT], f32, tag="av",
                                             name="av1")

                    st_ps = st_pool.tile([128, 2 * TT], f32, tag="st")
                    ssl = slice(s * 128, (s + 1) * 128)
                    nc.tensor.matmul(st_ps[:, 0:TT], k_sb[0:64, ssl],
                                     q_sb[0:64, tsl], start=True, stop=True)
                    nc.tensor.matmul(st_ps[:, TT:2 * TT], k_sb[64:128, ssl],
                                     q_sb[64:128, tsl], start=True, stop=True)

                    e_sb = e_pool.tile([128, 2 * TT], f16, tag="e")
                    # last tile: exp(29)->DVE, exp(30/31)->ACT so the flush's
                    # tail AVs wait ~1us instead of draining the DVE queue
                    use_act = act_pat[s]
                    if t == nt - 1 and s >= ns - 3:
                        use_act = (s >= ns - 2)
                    if use_act:
                        nc.scalar.activation(e_sb, st_ps, Exp, scale=SCALE)
                    else:
                        nc.vector._custom_dve(exp_op, out=e_sb, in0=st_ps,
                                              s0=EA, s1=EB, imm2=EK)
                    e_tiles[s] = e_sb

                    # AV runs 2 s-tiles behind QK so exp latency is hidden
                    if s >= 2:
                        sa = s - 2
                        ea = e_tiles.pop(sa)
                        nc.tensor.matmul(av[0][0:65, :], vt_sb[:, sa, 0:65],
                                         ea[:, 0:TT], start=(sa == 0),
                                         stop=False)
                        nc.tensor.matmul(av[1][0:65, :], vt_sb[:, sa, 65:130],
                                         ea[:, TT:2 * TT], start=(sa == 0),
                                         stop=False)

                # ---- epilogue closures, popped inside the NEXT tile ----
                a0, a1 = av
                e30, e31 = e_tiles.pop(ns - 2), e_tiles.pop(ns - 1)
                boxes = [None, None, None]  # ou, rz, rzb/o_sb

                def mk_tail(sa, ea, a0=a0, a1=a1):
                    def f():
                        sp = (sa == ns - 1)
                        nc.tensor.matmul(a0[0:65, :], vt_sb[:, sa, 0:65],
                                         ea[:, 0:TT], start=False, stop=sp)
                        nc.tensor.matmul(a1[0:65, :], vt_sb[:, sa, 65:130],
                                         ea[:, TT:2 * TT], start=False,
                                         stop=sp)
                    return f

                def mk_evac(a0=a0, a1=a1, bx=boxes, last=(t == nt - 1)):
                    def f():
                        # the unshifted ou half rides ACT (same pattern as the
                        # proven res copies); DVE keeps the shifted half and
                        # the z chain. In the final flush the z chain goes
                        # first: it gates the PE-side broadcast.
                        ou = o_pool.tile([128, TT], f32, tag="ou")

                        def emit_ou():
                            nc.scalar.activation(ou[0:64, :], a0[0:64, :],
                                                 Copy)
                            nc.vector.tensor_copy(ou[64:128, :], a1[0:64, :])

                        if not last:
                            emit_ou()
                        zr = z_pool.tile([1, 2, TT], f32, tag="zr")
                        nc.vector.tensor_copy(zr[0:1, 0, :], a0[64:65, :])
                        nc.vector.tensor_copy(zr[0:1, 1, :], a1[64:65, :])
                        # custom-DVE ops read garbage from non-zero PSUM
                        # partition bases, so the reciprocal runs on the SBUF
                        # staging copy (fp16 out feeds the fp16 broadcast)
                        rz = z_pool.tile([1, 2, TT], f16, tag="rz")
                        nc.vector._custom_dve(
                            RECIPROCAL_APPROX_FAST, out=rz, in0=zr,
                            s0=rcc["s0"], s1=rcc["s1"], imm2=rcc["imm2"])
                        if last:
                            emit_ou()
                        bx[0] = ou
                        bx[1] = (rz[0:1, 0, :], rz[0:1, 1, :])
                    return f

                def mk_b(bx=boxes):
                    def f():
                        rzb = pr_pool.tile([128, TT], f32, tag="pp")
                        nc.tensor.matmul(rzb, ones_a, bx[1][0],
                                         start=True, stop=False)
                        nc.tensor.matmul(rzb, ones_b, bx[1][1],
                                         start=False, stop=True)
                        bx[2] = rzb
                    return f

                def mk_norm(bx=boxes):
                    def f():
                        o_sb = o_pool.tile([128, TT], f16, tag="o")
                        nc.vector.tensor_mul(o_sb, bx[0], bx[2])
                        bx[2] = o_sb
                    return f

                def mk_proj(ot, tsl_t=tsl, bx=boxes, last=(t == nt - 1)):
                    def f():
                        # ot=1 borrows the av slot freed by the evacuation so
                        # consecutive projections double-buffer their psum
                        # (exactly one borrow per tile keeps the 3-slot av
                        # rotation collision-free; in the final flush all av
                        # slots are free so odd ots may borrow)
                        borrow = ot == 1 or (last and ot == 3)
                        pool = av_pool if borrow else pr_pool
                        pp = pool.tile([128, TT], f32,
                                       tag="av" if borrow else "pp", name="pp")
                        nc.tensor.matmul(pp, wp_r[:, ot * 128:(ot + 1) * 128],
                                         bx[2], start=True, stop=True)
                        res = res_pool.tile([128, TT], f16, tag="res")
                        nc.vector.tensor_copy(res, pp)
                        nc.sync.dma_start(
                            out=out[ot * 128:(ot + 1) * 128, tsl_t], in_=res)
                    return f

                pending = [(2, mk_tail(ns - 2, e30)), (2, mk_tail(ns - 1, e31)),
                           (2, mk_evac()), (4, mk_b()), (5, mk_norm())]
                for ot in range(KT):
                    pending.append((7 + 6 * ot, mk_proj(ot)))

            for _, f in pending:
                f()

    nc.compile()
    return nc


def _get_nc(l=L):
    if l not in _BUILT:
        _BUILT[l] = _build(l)
    return _BUILT[l]


def _ones2_pattern():
    o = np.zeros((1, 2, 128), dtype=np.float32)
    o[0, 0, 0:64] = 1.0
    o[0, 1, 64:128] = 1.0
    return o


def _shard_inputs(x, Wq, bq, Wkv, bkv, Wp, bp, l=L):
    x = np.asarray(x, dtype=np.float32)
    Wq = np.asarray(Wq, dtype=np.float32)
    bq = np.asarray(bq, dtype=np.float32)
    Wkv = np.asarray(Wkv, dtype=np.float32)
    bkv = np.asarray(bkv, dtype=np.float32)
    Wp = np.asarray(Wp, dtype=np.float32)

    in_maps = []
    for core in range(NCORES):
        b, hp = divmod(core, 4)
        sl = slice(hp * 128, (hp + 1) * 128)
        vsl = slice(C + hp * 128, C + (hp + 1) * 128)
        # w_all[p, i, kt, o]: SBUF layout — partition p, projection i (q|k|v),
        # contraction tile kt, out-channel o (this core's 128 channels)
        w_all = np.stack([Wq[sl, :].T, Wkv[sl, :].T, Wkv[vsl, :].T],
                         axis=1).reshape(KT, 128, 3, 128).transpose(1, 2, 0, 3)
        b_all = np.stack([bq[sl], bkv[sl], bkv[vsl]], axis=1)
        m = {
            "x": np.ascontiguousarray(
                x[b].reshape(KT, 128, l).astype(np.float16)),
            "w_all": np.ascontiguousarray(w_all.astype(np.float16)),
            "b_all": np.ascontiguousarray(b_all.astype(np.float32)),
            "wp": np.ascontiguousarray(Wp[:, sl].T.astype(np.float16)),
            "ones2": _ones2_pattern(),
        }
        in_maps.append(m)
    return in_maps


def _run(in_maps, l=L, trace=False):
    from concourse.bass_utils import run_bass_kernel_spmd
    nc = _get_nc(l)
    return run_bass_kernel_spmd(nc, in_maps, core_ids=list(range(NCORES)),
                                trace=trace)


def _gather(res, bp):
    outs = [res.results[i]["out"].astype(np.float32) for i in range(NCORES)]
    y = np.stack([outs[0] + outs[1] + outs[2] + outs[3],
                  outs[4] + outs[5] + outs[6] + outs[7]])
    y += np.asarray(bp, dtype=np.float32)[None, :, None]
    return np.ascontiguousarray(y.reshape(B, C, HH, WW), dtype=np.float32)


def kernel(x, Wq, bq, Wkv, bkv, Wp, bp):
    in_maps = _shard_inputs(x, Wq, bq, Wkv, bkv, Wp, bp)
    res = _run(in_maps)
    return _gather(res, bp)
